# revision 27
# baseline (speedup 1.0000x reference)
"""Trainium2 Bass kernel v3: per-image routed data augmentation (moe_routing).

For each image i, apply transform sample[i]:
  0: identity  1: fliplr  2: flipud  3: brightness(clip(1.5x))
  4: contrast(clip(1.5(x-mean)+mean))  5: solarize(x<0.5 ? x : 1-x)

Key identity: every transform is a two-piece linear function of v (the
loaded, possibly H-flipped tile) plus an optional W-reversed term:

    out = Relu( c * (Lrelu_a(-v + b) + B' * v_wrev) + D )

per-image scalars ([P,1] column APs; S = sum(v), m = S/PIX):
    t=0 identity:   a=1,  b=0,           c=-1,   B'=0,  D=0
    t=1 fliplr:     a=0,  b=0,           c=-1,   B'=-1, D=0
    t=2 flipud:     a=1,  b=0,           c=-1,   B'=0,  D=0   (flipped load)
    t=3 brightness: a=0,  b=2/3,         c=-1.5, B'=0,  D=1
    t=4 contrast:   a=0,  b=2/3+S/3PIX,  c=-1.5, B'=0,  D=1
    t=5 solarize:   a=-1, b=1/2,         c=-1,   B'=0,  D=1/2

Engine schedule per image (32 images/core, pure data parallel on 8 cores):
    DMA   predicated loads (flat layout: partition p = 1344 contiguous elems)
    DVE   scr = v (copy), accum_out -> row sums rs          (~0.7us, 2x mode)
    PE    sbc = ones^T @ rs   (partition reduce + broadcast)
    DVE   b_dyn = fb*sbc + bstat                            (tiny)
    ACT   g  = Prelu_alpha(-v + b_dyn)                      (alpha/bias APs)
    DVE   u3 = B'*v_wrev + g  (scalar_tensor_tensor)
    ACT   out = Relu(c*u3 + D)  (scale/bias APs)
    DMA   grouped store
"""

import numpy as np

import concourse.bass as bass
import concourse.bacc as bacc
import concourse.mybir as mybir
from concourse.tile import TileContext
from concourse.bass_utils import run_bass_kernel_spmd

N_CORES = 8
B = 256
B_LOC = B // N_CORES          # 32 images per core
C, H, W = 3, 224, 224
PIX = C * H * W               # 150528
P = 112                       # partitions
FREE = PIX // P               # 1344 contiguous elems per partition
Q = FREE // W                 # 6 W-rows per partition
GROUP = 4                     # images per store DMA

f32 = mybir.dt.float32
i32 = mybir.dt.int32
Alu = mybir.AluOpType
Act = mybir.ActivationFunctionType

_CACHE = {}


def _build_nc(repeat: int = 1, no_cond: bool = False):
    nc = bacc.Bacc()
    x = nc.declare_dram_parameter("x", [B_LOC, C, H, W], f32, isOutput=False)
    samp = nc.declare_dram_parameter("sample", [B_LOC], i32, isOutput=False)
    out = nc.declare_dram_parameter("out", [B_LOC, C, H, W], f32, isOutput=True)

    with TileContext(nc) as tc:
        with (
            tc.tile_pool(name="coef", bufs=1) as coef_pool,
            tc.tile_pool(name="data", bufs=4) as data_pool,
            tc.tile_pool(name="work", bufs=3) as work_pool,
            tc.tile_pool(name="outp", bufs=2) as out_pool,
            tc.tile_pool(name="stat", bufs=6) as stat_pool,
            tc.tile_pool(name="psum", bufs=4, space="PSUM") as psum_pool,
            tc.tile_pool(name="dram", bufs=4, space="DRAM") as dram_pool,
        ):

            def body():
                ones_t = coef_pool.tile([P, P], f32, tag="ones")
                nc.vector.memset(ones_t, 1.0)

                # ------- routing phase: per-image coefficient tables -------
                s_i = coef_pool.tile([1, B_LOC], i32)
                nc.sync.dma_start(s_i, samp[:].unsqueeze(0))
                s_f = coef_pool.tile([1, B_LOC], f32)
                nc.vector.tensor_copy(s_f, s_i)

                m = {}
                for k in (1, 3, 4, 5):
                    mk = coef_pool.tile([1, B_LOC], f32, tag=f"mask{k}")
                    nc.vector.tensor_scalar(mk, s_f, float(k), None, Alu.is_equal)
                    m[k] = mk
                m34 = coef_pool.tile([1, B_LOC], f32)
                nc.vector.tensor_tensor(m34, m[3], m[4], Alu.add)

                # a = 1 - m1 - m34 - 2*m5
                t1 = coef_pool.tile([1, B_LOC], f32, tag="t1")
                nc.vector.tensor_tensor(t1, m34, m[1], Alu.add)
                t2 = coef_pool.tile([1, B_LOC], f32, tag="t2")
                nc.vector.scalar_tensor_tensor(t2, m[5], 2.0, t1, Alu.mult, Alu.add)
                a_row = coef_pool.tile([1, B_LOC], f32)
                nc.vector.tensor_scalar(a_row, t2, -1.0, 1.0, Alu.mult, Alu.add)
                # bstat = (2/3)*m34 + 0.5*m5
                t3 = coef_pool.tile([1, B_LOC], f32, tag="t3")
                nc.vector.tensor_scalar(t3, m34, 2.0 / 3.0, None, Alu.mult)
                bstat_row = coef_pool.tile([1, B_LOC], f32)
                nc.vector.scalar_tensor_tensor(
                    bstat_row, m[5], 0.5, t3, Alu.mult, Alu.add)
                # fb = m4 / (3*PIX)
                fb_row = coef_pool.tile([1, B_LOC], f32)
                nc.vector.tensor_scalar(
                    fb_row, m[4], 1.0 / (3.0 * PIX), None, Alu.mult)
                # Bp = -m1
                Bp_row = coef_pool.tile([1, B_LOC], f32)
                nc.vector.tensor_scalar(Bp_row, m[1], -1.0, None, Alu.mult)
                # c = -1 - 0.5*m34
                c_row = coef_pool.tile([1, B_LOC], f32)
                nc.vector.tensor_scalar(c_row, m34, -0.5, -1.0, Alu.mult, Alu.add)
                # D = m34 + 0.5*m5
                D_row = coef_pool.tile([1, B_LOC], f32)
                nc.vector.scalar_tensor_tensor(
                    D_row, m[5], 0.5, m34, Alu.mult, Alu.add)

                # int flags for predicated loads
                is_ud = coef_pool.tile([1, B_LOC], i32)
                nc.vector.tensor_scalar(is_ud, s_i, 2, None, Alu.is_equal)
                not_ud = coef_pool.tile([1, B_LOC], i32)
                nc.vector.tensor_scalar(not_ud, s_i, 2, None, Alu.not_equal)

                # broadcast coefficient rows to all P partitions
                bc = {}
                for name, row in (
                    ("a", a_row), ("bstat", bstat_row), ("fb", fb_row),
                    ("Bp", Bp_row), ("c", c_row), ("D", D_row),
                ):
                    t = coef_pool.tile([P, B_LOC], f32, tag=f"bc_{name}")
                    nc.gpsimd.partition_broadcast(t, row)
                    bc[name] = t

                # ---------- main loop ----------
                og = None
                for i in range(B_LOC):
                    g_idx = i % GROUP
                    if g_idx == 0:
                        og = out_pool.tile([P, GROUP * FREE], f32, tag="og")

                    T = data_pool.tile([P, FREE], f32, tag="T")
                    src_n = x[i].flatten().rearrange("(p f) -> p f", p=P)
                    # unconditional normal load (static offset, fast path);
                    # flipud images are overwritten by the predicated reload
                    nc.sync.dma_start(T, src_n)
                    if not no_cond:
                        # UD path on the scalar engine's HWDGE ring so skipped
                        # DMAs don't serialize the main SP DMA ring
                        cond_u = nc.values_load(
                            is_ud[0:1, i:i + 1],
                            engines=(mybir.EngineType.Activation,),
                            min_val=0, max_val=1, skip_runtime_bounds_check=True)
                        xud_t = dram_pool.tile([C, H, W], f32, tag="xud")
                        nc.scalar.dma_start(xud_t[:], x[i, :, ::-1, :],
                                            cond=cond_u, cond_hint=False)
                        src_u = xud_t[:].flatten().rearrange("(p f) -> p f", p=P)
                        nc.scalar.dma_start(T, src_u, cond=cond_u, cond_hint=False)

                    T3 = T.rearrange("p (q w) -> p q w", q=Q)

                    a_col = bc["a"][:, i:i + 1]
                    bstat_col = bc["bstat"][:, i:i + 1]
                    fb_col = bc["fb"][:, i:i + 1]
                    Bp_col = bc["Bp"][:, i:i + 1]
                    c_col = bc["c"][:, i:i + 1]
                    D_col = bc["D"][:, i:i + 1]

                    # image sum via accum_out on a throwaway copy pass
                    scr = work_pool.tile([P, FREE], f32, tag="scr")
                    rs = stat_pool.tile([P, 1], f32, tag="rs")
                    nc.vector.tensor_scalar(
                        scr, T, 1.0, 0.0, Alu.mult, Alu.add, accum_out=rs)

                    # partition reduce + broadcast via idle-PE matmul with ones
                    sbc = psum_pool.tile([P, 1], f32, tag="sbc")
                    nc.tensor.matmul(sbc, ones_t, rs, start=True, stop=True)

                    b_dyn = stat_pool.tile([P, 1], f32, tag="b_dyn")
                    nc.vector.tensor_scalar(
                        b_dyn, sbc, fb_col, bstat_col, Alu.mult, Alu.add)

                    # g = Lrelu_a(-v + b)
                    g = work_pool.tile([P, FREE], f32, tag="g")
                    nc.scalar.activation(
                        g, T, Act.Prelu, bias=b_dyn, scale=-1.0, alpha=a_col)

                    # u3 = B' * v_wrev + g
                    u3 = work_pool.tile([P, FREE], f32, tag="u3")
                    T_wrev = T3[:, :, ::-1]
                    u3_3 = u3.rearrange("p (q w) -> p q w", q=Q)
                    g_3 = g.rearrange("p (q w) -> p q w", q=Q)
                    nc.vector.scalar_tensor_tensor(
                        u3_3, T_wrev, Bp_col, g_3, Alu.mult, Alu.add)

                    # out = Relu(c*u3 + D)
                    q = og[:, g_idx * FREE:(g_idx + 1) * FREE]
                    nc.scalar.activation(
                        q, u3, Act.Relu, bias=D_col, scale=c_col)

                    if g_idx == GROUP - 1:
                        i0 = i - (GROUP - 1)
                        dst = out[i0:i0 + GROUP].rearrange(
                            "b c h w -> b (c h w)").rearrange(
                            "b (p f) -> p b f", p=P)
                        og3 = og.rearrange("p (b f) -> p b f", b=GROUP)
                        nc.sync.dma_start(dst, og3)

            if repeat == 1:
                body()
            else:
                with tc.For_i(0, repeat, 1):
                    body()

    nc.compile()
    return nc


def kernel(x: np.ndarray, sample: np.ndarray) -> np.ndarray:
    x = np.ascontiguousarray(np.asarray(x, dtype=np.float32))
    sample = np.asarray(sample)
    if "nc" not in _CACHE:
        _CACHE["nc"] = _build_nc()
    nc = _CACHE["nc"]

    samp32 = np.ascontiguousarray(sample.astype(np.int32))
    in_maps = [
        {"x": x[i * B_LOC:(i + 1) * B_LOC], "sample": samp32[i * B_LOC:(i + 1) * B_LOC]}
        for i in range(N_CORES)
    ]
    res = run_bass_kernel_spmd(nc, in_maps, core_ids=list(range(N_CORES)))
    out = np.concatenate([r["out"] for r in res.results], axis=0)
    return out.astype(np.float32)
